# revision 11
# baseline (speedup 1.0000x reference)
"""BitNet linear layer (b1.58-style) on 8 Trainium2 NeuronCores.

Computes: scale = 1e-4 + mean(|W|); q = clip(round(W/scale), -1, 1);
          out = scale * (x @ q.T)
for x [4, 2048, 2048] f32 and W [8192, 2048] f32.

Sharding: tensor-parallel over out_features. Each core gets the full x
(replicated) and a 1024-row shard of the ternary q; cores run fully
independently and the host concatenates the per-core [8192, 1024]
output slices along the feature axis.

The elementwise prep is done once on the host (it is ~0.1% of the FLOPs
and would otherwise be redundantly recomputed per core): the exact
global scale and ternary q (bit-identical rounding vs the reference),
the f32->bf16 casts, and the transposes into SBUF-ready layouts.
`scale` is folded into the bf16 x cast, which is free in accuracy terms
(a single bf16 rounding either way), so the device applies no scale at
all. Remaining error is just the bf16 rounding of x (~2.3e-3).

The device is then a pure gap-free bf16 matmul at the PE roofline
(2048 matmuls of N=512 at ~216 ns cadence ~= 443 us):

  - xdev [8192, 2048] bf16 (replicated): row mt*128+p, col ko*128+m
    holds scale*x[token mt*128+m, k = ko*128+p] -- i.e. 64 m-tiles,
    each a [128k x (16ko x 128m)] stationary-operand block, 4 KiB
    contiguous per partition. One 512 KiB DMA per m-tile on the
    scalar queue, prefetched ~12 tiles deep.
  - qdev [128, 16384] bf16 (per-core shard): col ko*1024+n holds
    q[n-th row of shard, ko*128+p]. Loaded as 16 per-ko slices,
    even ko on the sync queue, odd ko on gpsimd, so the first
    matmul only waits for slice 0 and the rest land under compute.
  - Main loop over 64 m-tiles: x tile is the stationary operand
    (LDWEIGHTS hides under the moving stream), q is the moving
    operand; 16 k-steps of two 512-col accumulating matmuls into a
    psum bank pair (8 banks -> 4 m-tiles in flight). DVE drains
    psum -> f32 out tile; out rows stored in natural [M, N-shard]
    orientation on the sync queue.
"""

import sys

sys.path.insert(0, "/opt/trn_rl_repo")

import numpy as np
import ml_dtypes

import concourse.bass as bass
import concourse.tile as tile
from concourse import bacc, mybir
from concourse.bass_utils import run_bass_kernel_spmd

F32 = mybir.dt.float32
BF16 = mybir.dt.bfloat16
FP8 = mybir.dt.float8e4
BF16_NP = ml_dtypes.bfloat16
FP8_NP = ml_dtypes.float8_e4m3

NCORES = 8
M = 8192          # tokens (4*2048)
K = 2048          # in_features
N_FULL = 8192     # out_features
NS = N_FULL // NCORES  # 1024 per-core shard
P = 128
KO = K // P       # 16 k-tiles
MT = M // P       # 64 m-tiles


def build_nc():
    nc = bacc.Bacc("TRN2", target_bir_lowering=False, debug=False,
                   num_devices=NCORES)
    x_d = nc.dram_tensor("x", [M, K], BF16, kind="ExternalInput")
    qh_d = nc.dram_tensor("qh", [P, 2 * NS], BF16, kind="ExternalInput")
    qt_d = nc.dram_tensor("qt", [P, (KO - 2) * NS], FP8, kind="ExternalInput")
    o_d = nc.dram_tensor("out", [M, NS], BF16, kind="ExternalOutput")
    x_ap, qh_ap, qt_ap, o_ap = x_d.ap(), qh_d.ap(), qt_d.ap(), o_d.ap()

    with tile.TileContext(nc) as tc:
        with (
            tc.tile_pool(name="qpool", bufs=1) as qpool,
            tc.tile_pool(name="xpool", bufs=4) as xpool,
            tc.tile_pool(name="opool", bufs=4) as opool,
            tc.tile_pool(name="psum_o", bufs=8, space="PSUM") as psum_o,
        ):
            # ---- PE warmup --------------------------------------------
            # ~10 dummy matmuls on zeroed SBUF keep the PE busy through
            # the HAM SHORT window (~3.4us) while the first real data is
            # still in flight, so real matmuls start at 2.4 GHz instead
            # of paying the 1.2 GHz cold ramp.
            warm = qpool.tile([P, 640], BF16, name="warm")
            nc.scalar.memzero(warm[:])
            psW = psum_o.tile([P, 512], F32, name="psW", tag="ps")
            for _ in range(10):
                nc.tensor.matmul(psW[:], lhsT=warm[:, 0:P],
                                 rhs=warm[:, P:640], start=True, stop=True)

            # ---- resident ternary weights (moving operand) ------------
            # ko 0-1 land first as plain bf16 on the sync queue (ko 0 in
            # two 512-col granules so the very first matmul waits on only
            # 128 KiB); ko 2-15 ride the gpsimd queue as fp8 with
            # cast-during-DMA, halving their HBM bytes so the startup
            # burst clears sooner. The matmul always reads bf16 (an fp8
            # moving operand streams ~20% slower: 259 vs 216 ns/MM).
            tile_q = qpool.tile([P, KO * NS], BF16, name="q")
            for h in range(2):
                nc.sync.dma_start(tile_q[:, h * 512:(h + 1) * 512],
                                  qh_ap[:, h * 512:(h + 1) * 512])
            nc.sync.dma_start(tile_q[:, NS:2 * NS], qh_ap[:, NS:2 * NS])
            for g in range(7):
                ko = 2 + 2 * g
                nc.gpsimd.dma_start(
                    tile_q[:, ko * NS:(ko + 2) * NS],
                    qt_ap[:, (ko - 2) * NS:ko * NS])

            # ---- main loop: out[m, n] = sum_k x[m,k] q[n,k] -----------
            # m-tiles 0 and 1 are interleaved ko-major: during the
            # startup burst the q slices arrive at ~0.7us each, so
            # consuming each slice twice (0.86us) instead of once
            # (0.43us) keeps the PE fed instead of stalling ~8us.
            def x_load(mt, chunks):
                xt = xpool.tile([P, K], BF16, name=f"x_{mt}", tag="x")
                step = K // chunks
                for c in range(chunks):
                    nc.scalar.dma_start(
                        xt[:, c * step:(c + 1) * step],
                        x_ap[mt * P:(mt + 1) * P, c * step:(c + 1) * step])
                return xt

            def mm_pair(ps2, xt, ko):
                nc.tensor.matmul(
                    ps2[0][:], lhsT=xt[:, ko * P:(ko + 1) * P],
                    rhs=tile_q[:, ko * NS:ko * NS + 512],
                    start=(ko == 0), stop=(ko == KO - 1))
                nc.tensor.matmul(
                    ps2[1][:], lhsT=xt[:, ko * P:(ko + 1) * P],
                    rhs=tile_q[:, ko * NS + 512:(ko + 1) * NS],
                    start=(ko == 0), stop=(ko == KO - 1))

            def drain_store(mt, ps2):
                ot = opool.tile([P, NS], BF16, name=f"o_{mt}", tag="o")
                nc.vector.tensor_scalar(
                    ot[:, 0:512], ps2[0][:], 1.0, None, mybir.AluOpType.mult)
                nc.vector.tensor_scalar(
                    ot[:, 512:1024], ps2[1][:], 1.0, None,
                    mybir.AluOpType.mult)
                nc.sync.dma_start(o_ap[mt * P:(mt + 1) * P, :], ot[:])

            def ps_pair(mt):
                return (psum_o.tile([P, 512], F32, name=f"psA_{mt}", tag="ps"),
                        psum_o.tile([P, 512], F32, name=f"psB_{mt}", tag="ps"))

            xt0 = x_load(0, 4)
            xt1 = x_load(1, 4)
            ps0, ps1 = ps_pair(0), ps_pair(1)
            for ko in range(KO):
                mm_pair(ps0, xt0, ko)
                mm_pair(ps1, xt1, ko)
            drain_store(0, ps0)
            drain_store(1, ps1)

            for mt in range(2, MT):
                xt = x_load(mt, 1)
                ps = ps_pair(mt)
                for ko in range(KO):
                    mm_pair(ps, xt, ko)
                drain_store(mt, ps)

    nc.compile()
    return nc


_NC_CACHE = None


def get_nc():
    global _NC_CACHE
    if _NC_CACHE is None:
        _NC_CACHE = build_nc()
    return _NC_CACHE


def make_in_maps(x, weight):
    x2 = np.asarray(x, dtype=np.float32).reshape(M, K)
    w = np.asarray(weight, dtype=np.float32)

    # exact reference prep: scale from the full W, ternary q
    scale = np.float32(1e-4) + np.abs(w).mean(dtype=np.float32)
    q = np.clip(np.rint(w / scale), -1.0, 1.0).astype(np.float32)

    # xdev[mt*128+p, ko*128+m] = scale * x[mt*128+m, ko*128+p]
    xs = (x2 * scale).reshape(MT, P, KO, P)
    xdev = np.ascontiguousarray(
        xs.transpose(0, 3, 2, 1).reshape(M, K).astype(BF16_NP))

    # qdev_c[p, ko*1024+n] = q[c*1024+n, ko*128+p]  (ternary: exact in fp8)
    q4 = q.reshape(NCORES, NS, KO, P).transpose(0, 3, 2, 1)  # [c, p, ko, n]
    qdev = q4.reshape(NCORES, P, KO * NS)
    qh = np.ascontiguousarray(qdev[:, :, :2 * NS].astype(BF16_NP))
    qt = np.ascontiguousarray(qdev[:, :, 2 * NS:].astype(FP8_NP))

    return [{"x": xdev, "qh": qh[c], "qt": qt[c]} for c in range(NCORES)]


def kernel(x, weight):
    nc = get_nc()
    in_maps = make_in_maps(x, weight)
    try:
        res = run_bass_kernel_spmd(nc, in_maps, list(range(NCORES)))
    except Exception:
        # transient device errors have been observed on first touch; retry once
        res = run_bass_kernel_spmd(nc, in_maps, list(range(NCORES)))
    out = np.concatenate(
        [np.asarray(res.results[c]["out"]) for c in range(NCORES)], axis=1)
    return np.ascontiguousarray(out, dtype=np.float32).reshape(4, 2048, N_FULL)


# revision 16
# speedup vs baseline: 1.0081x; 1.0081x over previous
"""BitNet linear layer (b1.58-style) on 8 Trainium2 NeuronCores.

Computes: scale = 1e-4 + mean(|W|); q = clip(round(W/scale), -1, 1);
          out = scale * (x @ q.T)
for x [4, 2048, 2048] f32 and W [8192, 2048] f32.

Sharding: tensor-parallel over out_features. Each core gets the full x
(replicated) and a 1024-row shard of the ternary q; cores run fully
independently and the host concatenates the per-core [8192, 1024]
output slices along the feature axis.

The elementwise prep is done once on the host (it is ~0.1% of the FLOPs
and would otherwise be redundantly recomputed per core): the exact
global scale and ternary q (bit-identical rounding vs the reference),
the f32->bf16 casts, and the transposes into SBUF-ready layouts.
`scale` is folded into the bf16 x cast, which is free in accuracy terms
(a single bf16 rounding either way), so the device applies no scale at
all. Remaining error is just the bf16 rounding of x (~2.3e-3).

The device is then a pure gap-free bf16 matmul at the PE roofline
(2048 matmuls of N=512 at ~216 ns cadence ~= 443 us):

  - xdev [8192, 2048] bf16 (replicated): row mt*128+p, col ko*128+m
    holds scale*x[token mt*128+m, k = ko*128+p] -- i.e. 64 m-tiles,
    each a [128k x (16ko x 128m)] stationary-operand block, 4 KiB
    contiguous per partition. One 512 KiB DMA per m-tile on the
    scalar queue, prefetched ~12 tiles deep.
  - qdev [128, 16384] bf16 (per-core shard): col ko*1024+n holds
    q[n-th row of shard, ko*128+p]. Loaded as 16 per-ko slices,
    even ko on the sync queue, odd ko on gpsimd, so the first
    matmul only waits for slice 0 and the rest land under compute.
  - Main loop over 64 m-tiles: x tile is the stationary operand
    (LDWEIGHTS hides under the moving stream), q is the moving
    operand; 16 k-steps of two 512-col accumulating matmuls into a
    psum bank pair (8 banks -> 4 m-tiles in flight). DVE drains
    psum -> f32 out tile; out rows stored in natural [M, N-shard]
    orientation on the sync queue.
"""

import sys

sys.path.insert(0, "/opt/trn_rl_repo")

import numpy as np
import ml_dtypes

import concourse.bass as bass
import concourse.tile as tile
from concourse import bacc, mybir
from concourse.bass_utils import run_bass_kernel_spmd

F32 = mybir.dt.float32
BF16 = mybir.dt.bfloat16
FP8 = mybir.dt.float8e4
BF16_NP = ml_dtypes.bfloat16
FP8_NP = ml_dtypes.float8_e4m3

NCORES = 8
M = 8192          # tokens (4*2048)
K = 2048          # in_features
N_FULL = 8192     # out_features
NS = N_FULL // NCORES  # 1024 per-core shard
P = 128
KO = K // P       # 16 k-tiles
MT = M // P       # 64 m-tiles


def build_nc():
    nc = bacc.Bacc("TRN2", target_bir_lowering=False, debug=False,
                   num_devices=NCORES)
    x_d = nc.dram_tensor("x", [M, K], BF16, kind="ExternalInput")
    q_d = nc.dram_tensor("q", [P, KO * NS], FP8, kind="ExternalInput")
    o_d = nc.dram_tensor("out", [M, NS], BF16, kind="ExternalOutput")
    x_ap, q_ap, o_ap = x_d.ap(), q_d.ap(), o_d.ap()

    with tile.TileContext(nc) as tc:
        with (
            tc.tile_pool(name="qpool", bufs=1) as qpool,
            tc.tile_pool(name="qstage", bufs=3) as qstage,
            tc.tile_pool(name="xpool", bufs=6) as xpool,
            tc.tile_pool(name="opool", bufs=4) as opool,
            tc.tile_pool(name="psum_o", bufs=8, space="PSUM") as psum_o,
        ):
            # ---- PE warmup --------------------------------------------
            # ~10 dummy matmuls on zeroed SBUF keep the PE busy through
            # the HAM SHORT window (~3.4us) while the first real data is
            # still in flight, so real matmuls start at 2.4 GHz instead
            # of paying the 1.2 GHz cold ramp.
            warm = qpool.tile([P, 640], BF16, name="warm")
            nc.scalar.memzero(warm[:])
            psW = psum_o.tile([P, 512], F32, name="psW", tag="ps")
            for _ in range(10):
                nc.tensor.matmul(psW[:], lhsT=warm[:, 0:P],
                                 rhs=warm[:, P:640], start=True, stop=True)

            # ---- resident ternary weights (moving operand) ------------
            # q arrives as raw fp8 (2 MiB instead of 4: the SBUF-write
            # fabric is the startup bottleneck) in 2-ko slices on the
            # otherwise-idle sync queue, and is expanded fp8 -> bf16 by
            # the DVE (own SBUF ports, idle until the first drain). The
            # matmul always reads bf16 (an fp8 moving operand streams
            # ~20% slower: 259 vs 216 ns/MM measured).
            tile_q = qpool.tile([P, KO * NS], BF16, name="q")
            for g in range(KO // 2):
                ko = 2 * g
                qs = qstage.tile([P, 2 * NS], FP8, name=f"qs_{g}", tag="qs")
                nc.sync.dma_start(qs[:], q_ap[:, ko * NS:(ko + 2) * NS])
                nc.vector.tensor_scalar(
                    tile_q[:, ko * NS:(ko + 2) * NS], qs[:], 1.0, None,
                    mybir.AluOpType.mult)

            # ---- main loop: out[m, n] = sum_k x[m,k] q[n,k] -----------
            # m-tiles 0 and 1 are interleaved ko-major: during the
            # startup burst the q slices arrive at ~0.7us each, so
            # consuming each slice twice (0.86us) instead of once
            # (0.43us) keeps the PE fed instead of stalling ~8us.
            def x_load(mt, chunks, eng=None):
                eng = eng or nc.scalar
                xt = xpool.tile([P, K], BF16, name=f"x_{mt}", tag="x")
                step = K // chunks
                for c in range(chunks):
                    eng.dma_start(
                        xt[:, c * step:(c + 1) * step],
                        x_ap[mt * P:(mt + 1) * P, c * step:(c + 1) * step])
                return xt

            def mm_pair(ps2, xt, ko):
                nc.tensor.matmul(
                    ps2[0][:], lhsT=xt[:, ko * P:(ko + 1) * P],
                    rhs=tile_q[:, ko * NS:ko * NS + 512],
                    start=(ko == 0), stop=(ko == KO - 1))
                nc.tensor.matmul(
                    ps2[1][:], lhsT=xt[:, ko * P:(ko + 1) * P],
                    rhs=tile_q[:, ko * NS + 512:(ko + 1) * NS],
                    start=(ko == 0), stop=(ko == KO - 1))

            def drain_store(mt, ps2):
                ot = opool.tile([P, NS], BF16, name=f"o_{mt}", tag="o")
                nc.vector.tensor_scalar(
                    ot[:, 0:512], ps2[0][:], 1.0, None, mybir.AluOpType.mult)
                nc.vector.tensor_scalar(
                    ot[:, 512:1024], ps2[1][:], 1.0, None,
                    mybir.AluOpType.mult)
                nc.sync.dma_start(o_ap[mt * P:(mt + 1) * P, :], ot[:])

            def ps_pair(mt):
                return (psum_o.tile([P, 512], F32, name=f"psA_{mt}", tag="ps"),
                        psum_o.tile([P, 512], F32, name=f"psB_{mt}", tag="ps"))

            # m-tiles 0-1 ride the gpsimd queue (free of other traffic at
            # startup) in 4 chunks each; the scalar queue starts on mt 2
            # immediately so the steady-state x feed is never behind.
            xt0 = x_load(0, 4, nc.gpsimd)
            xt1 = x_load(1, 4, nc.gpsimd)
            ps0, ps1 = ps_pair(0), ps_pair(1)
            for ko in range(KO):
                mm_pair(ps0, xt0, ko)
                mm_pair(ps1, xt1, ko)
            drain_store(0, ps0)
            drain_store(1, ps1)

            for mt in range(2, MT - 1):
                xt = x_load(mt, 1)
                ps = ps_pair(mt)
                for ko in range(KO):
                    mm_pair(ps, xt, ko)
                drain_store(mt, ps)

            # last m-tile: run the psA sweep to completion first so its
            # drain+store overlap the psB sweep, shortening the tail
            mt = MT - 1
            xt = x_load(mt, 1)
            psA, psB = ps_pair(mt)
            for ko in range(KO):
                nc.tensor.matmul(
                    psA[:], lhsT=xt[:, ko * P:(ko + 1) * P],
                    rhs=tile_q[:, ko * NS:ko * NS + 512],
                    start=(ko == 0), stop=(ko == KO - 1))
            ot = opool.tile([P, NS], BF16, name=f"o_{mt}", tag="o")
            nc.vector.tensor_scalar(
                ot[:, 0:512], psA[:], 1.0, None, mybir.AluOpType.mult)
            nc.sync.dma_start(o_ap[mt * P:(mt + 1) * P, 0:512], ot[:, 0:512])
            for ko in range(KO):
                nc.tensor.matmul(
                    psB[:], lhsT=xt[:, ko * P:(ko + 1) * P],
                    rhs=tile_q[:, ko * NS + 512:(ko + 1) * NS],
                    start=(ko == 0), stop=(ko == KO - 1))
            nc.vector.tensor_scalar(
                ot[:, 512:1024], psB[:], 1.0, None, mybir.AluOpType.mult)
            nc.sync.dma_start(o_ap[mt * P:(mt + 1) * P, 512:1024],
                              ot[:, 512:1024])

    nc.compile()
    return nc


_NC_CACHE = None


def get_nc():
    global _NC_CACHE
    if _NC_CACHE is None:
        _NC_CACHE = build_nc()
    return _NC_CACHE


def make_in_maps(x, weight):
    x2 = np.asarray(x, dtype=np.float32).reshape(M, K)
    w = np.asarray(weight, dtype=np.float32)

    # exact reference prep: scale from the full W, ternary q
    scale = np.float32(1e-4) + np.abs(w).mean(dtype=np.float32)
    q = np.clip(np.rint(w / scale), -1.0, 1.0).astype(np.float32)

    # xdev[mt*128+p, ko*128+m] = scale * x[mt*128+m, ko*128+p]
    xs = (x2 * scale).reshape(MT, P, KO, P)
    xdev = np.ascontiguousarray(
        xs.transpose(0, 3, 2, 1).reshape(M, K).astype(BF16_NP))

    # qdev_c[p, ko*1024+n] = q[c*1024+n, ko*128+p]  (ternary: exact in fp8)
    q4 = q.reshape(NCORES, NS, KO, P).transpose(0, 3, 2, 1)  # [c, p, ko, n]
    qdev = np.ascontiguousarray(q4.reshape(NCORES, P, KO * NS).astype(FP8_NP))

    return [{"x": xdev, "q": qdev[c]} for c in range(NCORES)]


def kernel(x, weight):
    nc = get_nc()
    in_maps = make_in_maps(x, weight)
    try:
        res = run_bass_kernel_spmd(nc, in_maps, list(range(NCORES)))
    except Exception:
        # transient device errors have been observed on first touch; retry once
        res = run_bass_kernel_spmd(nc, in_maps, list(range(NCORES)))
    out = np.concatenate(
        [np.asarray(res.results[c]["out"]) for c in range(NCORES)], axis=1)
    return np.ascontiguousarray(out, dtype=np.float32).reshape(4, 2048, N_FULL)
